# revision 16
# baseline (speedup 1.0000x reference)
"""CRF loss (multi-annotator) Trainium2 kernel.

Problem (hardcoded): scores (8,200,64,32,32) f32, targets (8,200,64) int,
mask (200,64) bool, a_mask (8,64) bool -> scalar f32 loss.

Sharding: one annotator per NeuronCore (8 cores). Each core computes
losses[b] = log_Z[b] - tg_energy[b] for its annotator; host applies a_mask
and sums / B.

Device algorithm per core:
  - scores stream in s-blocks; SBUF tile rows p = h*64+b (h = tf half),
    free = (j=tf%16, tt) contiguous 2KB rows; exp on ACT reads the (tt,j)
    transposed view and writes (tt,j)-ordered bf16.
  - no-log forward scan, state wsp[h*64+b, j] = exp(p_scan[b, h*16+j] - C[b])
    in bf16. Per step:
      y = exp(scores) * wsp (one TT, wsp broadcast over tt)
      sh = reduce_j y (bf16 tree adds)
      pt(PSUM) = dup128.T @ sh  (PE: adds the two h-halves AND duplicates the
                                 full tt vector into both halves - the only
                                 cross-partition op)
      wsp halves <- pt column slices (one copy on DVE, one on ScalarE)
    Renorm every RENORM steps: m = rowmax(pt) (valid in all rows), wsp *= 1/m,
    m stored to mxbuf; ALL logs deferred to one end-of-kernel pass
    (C prefix sums recovered with a host-built window mask).
  - mask via capture-at-cutoff: hit[s,b] = (s == len_b-1); for s >= S//2-1:
    cap_end += hit * pt[0:64, END_TAG].
  - tg_energy: dma_gather of the 256B block holding each target score,
    then one fused (gather * onehot_mask) row-sum pass; halves combined by
    the same PE duplicate trick.
"""

import os
import sys

import numpy as np

if os.path.isdir("/opt/trn_rl_repo"):
    sys.path.insert(0, "/opt/trn_rl_repo")

import ml_dtypes  # noqa: E402

import concourse.bass as bass  # noqa: E402
import concourse.tile as tile  # noqa: E402
from concourse import bacc, mybir  # noqa: E402
from concourse.bass_utils import run_bass_kernel_spmd  # noqa: E402

F32 = mybir.dt.float32
BF16 = mybir.dt.bfloat16
I16 = mybir.dt.int16

A, S, B, T = 8, 200, 64, 32
START_TAG, END_TAG = 30, 31
SBLK = 16     # steps per streamed DMA block
GBLK = 16     # steps per dma_gather chunk
RENORM = 8    # renorm period (steps)


def _plan(S):
    """Gather chunk plan: list of (s0, nsteps, idx_col0, out_col0)."""
    chunks = []
    s0 = 0
    idx_col = 0
    out_col = 0
    while s0 < S:
        ns = min(GBLK, S - s0)
        ni = ns * B
        assert ni % 128 == 0
        chunks.append((s0, ns, idx_col, out_col))
        idx_col += ni // 16
        out_col += ni // 128
        s0 += ns
    return chunks, idx_col, out_col


def _n_renorms(S):
    # renorms at s = RENORM, 2*RENORM, ... <= S-2
    return max(0, (S - 2) // RENORM)


def build_nc(S=S):
    """Build the Bass/Tile program (same for all cores)."""
    from contextlib import ExitStack

    chunks, idx_cols, out_blocks = _plan(S)
    NR = _n_renorms(S)
    smin = S // 2 - 1  # earliest possible hit step (lens >= S//2)

    nc = bacc.Bacc("TRN2", target_bir_lowering=False, debug=False, num_devices=8)

    sc_d = nc.dram_tensor("sc", [S, B, T, T], F32, kind="ExternalInput").ap()
    gidx_d = nc.dram_tensor("gidx", [128, idx_cols], I16, kind="ExternalInput").ap()
    oh_d = nc.dram_tensor("oh", [128, out_blocks * 64], F32, kind="ExternalInput").ap()
    hit_d = nc.dram_tensor("hit", [64, S], F32, kind="ExternalInput").ap()
    hcum_d = nc.dram_tensor("hcum", [64, NR], F32, kind="ExternalInput").ap()
    dup_d = nc.dram_tensor("dup", [128, 128], BF16, kind="ExternalInput").ap()
    out_d = nc.dram_tensor("losses", [64, 1], F32, kind="ExternalOutput").ap()

    with tile.TileContext(nc) as tc, ExitStack() as ctx:
        state = ctx.enter_context(tc.tile_pool(name="state", bufs=1))
        blocks = ctx.enter_context(tc.tile_pool(name="blocks", bufs=3))
        work = ctx.enter_context(tc.tile_pool(name="work", bufs=4))
        gathp = ctx.enter_context(tc.tile_pool(name="gath", bufs=1))
        psum = ctx.enter_context(tc.tile_pool(name="psum", bufs=2, space="PSUM"))

        # ---- persistent state ----
        wsp = state.tile([128, 16], BF16)        # exp(p - C), compact halves
        cap_end = state.tile([64, 1], F32)
        mxbuf = state.tile([64, NR], F32)        # renorm maxes (logs deferred)
        hit = state.tile([64, S], F32)
        hcum = state.tile([64, NR], F32)
        dup = state.tile([128, 128], BF16)
        gath = gathp.tile([128, out_blocks * 64], F32)
        oh = gathp.tile([128, out_blocks * 64], F32)
        gidx = gathp.tile([128, idx_cols], I16)

        nc.sync.dma_start(hit[:], hit_d[:])
        nc.sync.dma_start(hcum[:], hcum_d[:])
        nc.sync.dma_start(gidx[:], gidx_d[:])
        nc.sync.dma_start(oh[:], oh_d[:])
        nc.sync.dma_start(dup[:], dup_d[:])

        nc.vector.memset(cap_end[:], 0.0)

        # ---- gather chunks (overlap with scan) ----
        for (s0, ns, icol, ocol) in chunks:
            ni = ns * B
            src = sc_d[s0 : s0 + ns].rearrange("s b f t -> (s b f t)")
            src_blk = src.rearrange("(n e) -> n e", e=64)
            nc.gpsimd.dma_gather(
                gath[:, ocol * 64 : (ocol + ni // 128) * 64].rearrange(
                    "p (c e) -> p c e", e=64
                ),
                src_blk,
                gidx[:, icol : icol + ni // 16],
                num_idxs=ni,
                num_idxs_reg=ni,
                elem_size=64,
            )

        # ---- streamed score blocks ----
        def load_block(bi):
            s0 = bi * SBLK
            ns = min(SBLK, S - s0)
            blk = blocks.tile([128, SBLK * 512], F32, tag="blk")
            for h in range(2):
                src = (
                    sc_d[s0 : s0 + ns, :, h * 16 : (h + 1) * 16, :]
                    .transpose([1, 0, 2, 3])
                    .rearrange("b s j t -> b s (j t)")
                )
                dst = blk[h * 64 : (h + 1) * 64, 0 : ns * 512].rearrange(
                    "p (s f) -> p s f", s=ns
                )
                nc.sync.dma_start(dst, src)
            return blk

        def dup_combine(sh_ap, tag, n):
            """PSUM <- dup.T @ sh : out[p] = sh[p%64] + sh[64 + p%64]."""
            pt = psum.tile([128, n], F32, tag=tag)
            nc.tensor.matmul(pt[:], dup[:], sh_ap, start=True, stop=True)
            return pt

        def wsp_update(pt):
            # compact halves: wsp[(h,b), j] = pt[(h,b), h*16+j]
            nc.vector.tensor_copy(wsp[0:64, :], pt[0:64, 0:16])
            nc.vector.tensor_copy(wsp[64:128, :], pt[64:128, 16:32])

        blk = load_block(0)

        # ---- init from step 0: p0 = scores[0, :, START_TAG, :] ----
        # START_TAG=30 -> h=1, j=14: rows 64:128, cols 14*32..15*32
        sh = state.tile([128, 32], BF16)  # per-step half sums
        nc.vector.memset(sh[0:64, :], 0.0)
        nc.scalar.activation(
            sh[64:128, :], blk[64:128, 448:480], mybir.ActivationFunctionType.Exp
        )
        pt = dup_combine(sh[:], "wfull", 32)
        wsp_update(pt)

        # ---- main scan ----
        nren = 0
        for s in range(1, S):
            bi, sl = divmod(s, SBLK)
            if sl == 0:
                blk = load_block(bi)
            R = blk[:, sl * 512 : (sl + 1) * 512]
            # exp: read (j,t) storage via (t,j) view; write (t,j)-ordered bf16
            e16 = work.tile([128, 512], BF16, tag="e16")
            nc.scalar.activation(
                e16[:].rearrange("p (t j) -> p t j", j=16),
                R.rearrange("p (j t) -> p t j", j=16),
                mybir.ActivationFunctionType.Exp,
            )
            # y = e * w : wsp row slice broadcast over tt (single 128-row op)
            y16 = work.tile([128, 512], BF16, tag="y16")
            nc.vector.tensor_mul(
                y16[:].rearrange("p (t j) -> p t j", j=16),
                e16[:].rearrange("p (t j) -> p t j", j=16),
                wsp[:].unsqueeze(1).broadcast_to([128, 32, 16]),
            )
            # reduce over j (innermost, contiguous); bf16 out feeds the matmul
            with nc.allow_low_precision("sum of 16 bf16 terms; fp32 internal"):
                nc.vector.reduce_sum(
                    sh[:],
                    y16[:].rearrange("p (t j) -> p t j", j=16),
                    axis=mybir.AxisListType.X,
                )
            # cross-partition combine + duplicate on PE
            pt = dup_combine(sh[:], "wfull", 32)
            wsp_update(pt)

            # capture (only steps where a cutoff can occur); pt has full tt
            if s >= smin:
                nc.vector.scalar_tensor_tensor(
                    cap_end[:],
                    pt[0:64, END_TAG : END_TAG + 1],
                    hit[:, s : s + 1],
                    cap_end[:],
                    op0=mybir.AluOpType.mult,
                    op1=mybir.AluOpType.add,
                )
            # renorm (logs deferred): every RENORM steps, not at the end
            if (s % RENORM) == 0 and s <= S - 2:
                mx = work.tile([128, 1], F32, tag="mx")
                nc.vector.reduce_max(mx[:], pt[:], axis=mybir.AxisListType.X)
                nc.vector.tensor_copy(mxbuf[:, nren : nren + 1], mx[0:64, :])
                rcp = work.tile([128, 1], F32, tag="rcp")
                nc.vector.reciprocal(rcp[:], mx[:])
                nc.vector.tensor_scalar_mul(wsp[:], wsp[:], rcp[:])
                nren += 1
        assert nren == NR, (nren, NR)

        # ---- tg energy: one fused pass over gathered blocks ----
        tgacc = state.tile([128, 1], F32)
        tgtmp = gathp.tile([128, out_blocks * 64], F32)
        nc.vector.scalar_tensor_tensor(
            tgtmp[:],
            gath[:],
            1.0,
            oh[:],
            op0=mybir.AluOpType.mult,
            op1=mybir.AluOpType.mult,
            accum_out=tgacc[:],
        )
        # tg halves combined with an f32 dup matmul (precision)
        ptg = psum.tile([128, 1], F32, tag="tg")
        dupf = state.tile([128, 128], F32)
        nc.vector.tensor_copy(dupf[:], dup[:])
        nc.tensor.matmul(ptg[:], dupf[:], tgacc[:], start=True, stop=True)

        # ---- deferred logs + loss assembly ----
        # cap_C[b] = sum_q hcum[b,q] * ln(mxbuf[b,q])
        lnmx = state.tile([64, NR], F32)
        nc.scalar.activation(lnmx[:], mxbuf[:], mybir.ActivationFunctionType.Ln)
        capCtmp = state.tile([64, NR], F32)
        cap_C = state.tile([64, 1], F32)
        nc.vector.scalar_tensor_tensor(
            capCtmp[:],
            lnmx[:],
            1.0,
            hcum[:],
            op0=mybir.AluOpType.mult,
            op1=mybir.AluOpType.mult,
            accum_out=cap_C[:],
        )
        lw = state.tile([64, 1], F32)
        nc.scalar.activation(lw[:], cap_end[:], mybir.ActivationFunctionType.Ln)
        res = state.tile([64, 1], F32)
        nc.vector.tensor_add(res[:], cap_C[:], lw[:])
        nc.vector.tensor_sub(res[:], res[:], ptg[0:64, :])
        nc.sync.dma_start(out_d[:], res[:])

    nc.compile()
    return nc


def host_prep(targets_a: np.ndarray, mask: np.ndarray, S=S):
    """Per-annotator index tensors (pure index arithmetic on targets/mask)."""
    chunks, idx_cols, out_blocks = _plan(S)
    NR = _n_renorms(S)

    tgt = targets_a.astype(np.int64)  # (S, B)
    maskf = mask.astype(np.float32)  # (S, B)
    lens = mask.astype(np.int64).sum(axis=0)  # (B,)
    assert lens.min() >= S // 2, "kernel assumes valid-prefix lens >= S//2"

    hit = np.zeros((64, S), dtype=np.float32)
    hcum = np.zeros((64, NR), dtype=np.float32)
    for b in range(B):
        sb = int(lens[b]) - 1
        hit[b, sb] = 1.0
        win = (sb - 1) // RENORM
        hcum[b, : min(win, NR)] = 1.0

    gidx = np.zeros((128, idx_cols), dtype=np.int16)
    oh = np.zeros((128, out_blocks * 64), dtype=np.float32)
    ohv = oh.reshape(128, out_blocks, 64)
    for (s0, ns, icol, ocol) in chunks:
        ni = ns * B
        i = np.arange(ni)
        sl, bb = np.divmod(i, B)
        rel = (sl * B + bb) * (T * T) + tgt[s0 + sl, bb]
        blk, e = np.divmod(rel, 64)
        gidx[i % 16, icol + i // 16] = blk.astype(np.int16)
        ohv[i % 128, ocol + i // 128, e] = maskf[s0 + sl, bb]
    for g in range(1, 8):
        gidx[16 * g : 16 * (g + 1)] = gidx[:16]

    p = np.arange(128)
    m = np.arange(128)
    dup = ((p[:, None] % 64) == (m[None, :] % 64)).astype(ml_dtypes.bfloat16)

    return dict(gidx=gidx, oh=oh, hit=hit, hcum=hcum, dup=dup)


_NC_CACHE = {}

# Set by test harness to capture profiles; harmless defaults for grading.
TRACE = False
TRACE_DIR = None
LAST_RESULTS = None


def _get_nc(S=S):
    if S not in _NC_CACHE:
        _NC_CACHE[S] = build_nc(S)
    return _NC_CACHE[S]


def kernel(scores, targets, mask, a_mask):
    scores = np.asarray(scores)
    targets = np.asarray(targets)
    mask_np = np.asarray(mask).astype(bool)
    a_mask_np = np.asarray(a_mask).astype(bool)

    nc = _get_nc(scores.shape[1])

    in_maps = []
    for a in range(A):
        prep = host_prep(targets[a], mask_np, S=scores.shape[1])
        m = dict(sc=np.ascontiguousarray(scores[a]), **prep)
        in_maps.append(m)

    if TRACE:
        # Profiling-only: the image's antenv package (imported at boot from a
        # read-only path) lacks axon_hooks; point it at our shim so
        # bass_utils' trace path can find the NTFF hook.
        import antenv

        shim = "/opt/trn_rl_repo/antenv"
        if shim not in list(antenv.__path__):
            antenv.__path__.append(shim)

    global LAST_RESULTS
    res = run_bass_kernel_spmd(
        nc, in_maps, core_ids=list(range(A)), trace=TRACE, tmpdir=TRACE_DIR
    )
    LAST_RESULTS = res
    losses = np.stack([r["losses"][:, 0] for r in res.results])  # (A, B)
    loss = np.where(a_mask_np, losses, 0.0).sum(dtype=np.float32) / np.float32(B)
    return np.float32(loss)


# revision 19
# speedup vs baseline: 1.0412x; 1.0412x over previous
"""CRF loss (multi-annotator) Trainium2 kernel.

Problem (hardcoded): scores (8,200,64,32,32) f32, targets (8,200,64) int,
mask (200,64) bool, a_mask (8,64) bool -> scalar f32 loss.

Sharding: one annotator per NeuronCore (8 cores). Each core computes
losses[b] = log_Z[b] - tg_energy[b] for its annotator; host applies a_mask
and sums / B.

Device algorithm per core:
  - scores stream in s-blocks; SBUF tile rows p = h*64+b (h = tf half),
    free = (j=tf%16, tt) contiguous 2KB rows; exp on ACT reads the (tt,j)
    transposed view and writes (tt,j)-ordered bf16.
  - no-log forward scan, state wsp[h*64+b, j] = exp(p_scan[b, h*16+j] - C[b])
    in bf16. Per step:
      y = exp(scores) * wsp (one TT, wsp broadcast over tt)
      sh = reduce_j y (bf16 tree adds)
      pt(PSUM) = dup128.T @ sh  (PE: adds the two h-halves AND duplicates the
                                 full tt vector into both halves - the only
                                 cross-partition op)
      wsp halves <- pt column slices (one copy on DVE, one on ScalarE)
    Renorm every RENORM steps: m = rowmax(pt) (valid in all rows), wsp *= 1/m,
    m stored to mxbuf; ALL logs deferred to one end-of-kernel pass
    (C prefix sums recovered with a host-built window mask).
  - mask via capture-at-cutoff: hit[s,b] = (s == len_b-1); for s >= S//2-1:
    cap_end += hit * pt[0:64, END_TAG].
  - tg_energy: dma_gather of the 256B block holding each target score,
    then one fused (gather * onehot_mask) row-sum pass; halves combined by
    the same PE duplicate trick.
"""

import os
import sys

import numpy as np

if os.path.isdir("/opt/trn_rl_repo"):
    sys.path.insert(0, "/opt/trn_rl_repo")

import ml_dtypes  # noqa: E402

import concourse.bass as bass  # noqa: E402
import concourse.tile as tile  # noqa: E402
from concourse import bacc, mybir  # noqa: E402
from concourse.tile_rust import add_dep_helper  # noqa: E402
from concourse.bass_utils import run_bass_kernel_spmd  # noqa: E402

F32 = mybir.dt.float32
BF16 = mybir.dt.bfloat16
I16 = mybir.dt.int16

A, S, B, T = 8, 200, 64, 32
START_TAG, END_TAG = 30, 31
SBLK = 16     # steps per streamed DMA block
GBLK = 16     # steps per dma_gather chunk
RENORM = 8    # renorm period (steps)


def _plan(S):
    """Gather chunk plan: list of (s0, nsteps, idx_col0, out_col0)."""
    chunks = []
    s0 = 0
    idx_col = 0
    out_col = 0
    while s0 < S:
        ns = min(GBLK, S - s0)
        ni = ns * B
        assert ni % 128 == 0
        chunks.append((s0, ns, idx_col, out_col))
        idx_col += ni // 16
        out_col += ni // 128
        s0 += ns
    return chunks, idx_col, out_col


def _n_renorms(S):
    # renorms at s = RENORM, 2*RENORM, ... <= S-2
    return max(0, (S - 2) // RENORM)


def build_nc(S=S):
    """Build the Bass/Tile program (same for all cores)."""
    from contextlib import ExitStack

    chunks, idx_cols, out_blocks = _plan(S)
    NR = _n_renorms(S)
    smin = S // 2 - 1  # earliest possible hit step (lens >= S//2)

    nc = bacc.Bacc("TRN2", target_bir_lowering=False, debug=False, num_devices=8)

    sc_d = nc.dram_tensor("sc", [S, B, T, T], F32, kind="ExternalInput").ap()
    gidx_d = nc.dram_tensor("gidx", [128, idx_cols], I16, kind="ExternalInput").ap()
    oh_d = nc.dram_tensor("oh", [128, out_blocks * 64], F32, kind="ExternalInput").ap()
    hit_d = nc.dram_tensor("hit", [64, S], F32, kind="ExternalInput").ap()
    hcum_d = nc.dram_tensor("hcum", [64, NR], F32, kind="ExternalInput").ap()
    dup_d = nc.dram_tensor("dup", [128, 128], BF16, kind="ExternalInput").ap()
    out_d = nc.dram_tensor("losses", [64, 1], F32, kind="ExternalOutput").ap()

    with tile.TileContext(nc) as tc, ExitStack() as ctx:
        state = ctx.enter_context(tc.tile_pool(name="state", bufs=1))
        blocks = ctx.enter_context(tc.tile_pool(name="blocks", bufs=3))
        work = ctx.enter_context(tc.tile_pool(name="work", bufs=4))
        gathp = ctx.enter_context(tc.tile_pool(name="gath", bufs=1))
        psum = ctx.enter_context(tc.tile_pool(name="psum", bufs=2, space="PSUM"))

        # ---- persistent state ----
        wsp = state.tile([128, 16], BF16)        # exp(p - C), compact halves
        cap_end = state.tile([64, 1], F32)
        mxbuf = state.tile([64, NR], F32)        # renorm maxes (logs deferred)
        hit = state.tile([64, S], F32)
        hcum = state.tile([64, NR], F32)
        dup = state.tile([128, 128], BF16)
        gath = gathp.tile([128, out_blocks * 64], F32)
        oh = gathp.tile([128, out_blocks * 64], F32)
        gidx = gathp.tile([128, idx_cols], I16)

        nc.sync.dma_start(hit[:], hit_d[:])
        nc.sync.dma_start(hcum[:], hcum_d[:])
        nc.sync.dma_start(gidx[:], gidx_d[:])
        nc.sync.dma_start(oh[:], oh_d[:])
        nc.sync.dma_start(dup[:], dup_d[:])

        nc.vector.memset(cap_end[:], 0.0)

        # ---- gather chunks (overlap with scan) ----
        for (s0, ns, icol, ocol) in chunks:
            ni = ns * B
            src = sc_d[s0 : s0 + ns].rearrange("s b f t -> (s b f t)")
            src_blk = src.rearrange("(n e) -> n e", e=64)
            nc.gpsimd.dma_gather(
                gath[:, ocol * 64 : (ocol + ni // 128) * 64].rearrange(
                    "p (c e) -> p c e", e=64
                ),
                src_blk,
                gidx[:, icol : icol + ni // 16],
                num_idxs=ni,
                num_idxs_reg=ni,
                elem_size=64,
            )

        # ---- streamed score blocks ----
        def load_block(bi):
            s0 = bi * SBLK
            ns = min(SBLK, S - s0)
            blk = blocks.tile([128, SBLK * 512], F32, tag="blk")
            for h in range(2):
                src = (
                    sc_d[s0 : s0 + ns, :, h * 16 : (h + 1) * 16, :]
                    .transpose([1, 0, 2, 3])
                    .rearrange("b s j t -> b s (j t)")
                )
                dst = blk[h * 64 : (h + 1) * 64, 0 : ns * 512].rearrange(
                    "p (s f) -> p s f", s=ns
                )
                nc.sync.dma_start(dst, src)
            return blk

        def dup_combine(sh_ap, tag, n):
            """PSUM <- dup.T @ sh : out[p] = sh[p%64] + sh[64 + p%64]."""
            pt = psum.tile([128, n], F32, tag=tag)
            nc.tensor.matmul(pt[:], dup[:], sh_ap, start=True, stop=True)
            return pt

        def wsp_update(pt):
            # compact halves: wsp[(h,b), j] = pt[(h,b), h*16+j]
            nc.vector.tensor_copy(wsp[0:64, :], pt[0:64, 0:16])
            nc.scalar.copy(wsp[64:128, :], pt[64:128, 16:32])

        blk = load_block(0)

        # ---- init from step 0: p0 = scores[0, :, START_TAG, :] ----
        # START_TAG=30 -> h=1, j=14: rows 64:128, cols 14*32..15*32
        sh = state.tile([128, 32], BF16)  # per-step half sums
        nc.vector.memset(sh[0:64, :], 0.0)
        nc.scalar.activation(
            sh[64:128, :], blk[64:128, 448:480], mybir.ActivationFunctionType.Exp
        )
        pt = dup_combine(sh[:], "wfull", 32)
        wsp_update(pt)

        # ---- main scan ----
        nren = 0
        for s in range(1, S):
            bi, sl = divmod(s, SBLK)
            if sl == 0:
                blk = load_block(bi)
            R = blk[:, sl * 512 : (sl + 1) * 512]
            # exp: read (j,t) storage via (t,j) view; write (t,j)-ordered bf16
            e16 = work.tile([128, 512], BF16, tag="e16")
            nc.scalar.activation(
                e16[:].rearrange("p (t j) -> p t j", j=16),
                R.rearrange("p (j t) -> p t j", j=16),
                mybir.ActivationFunctionType.Exp,
            )
            # y = e * w : wsp row slice broadcast over tt (single 128-row op)
            y16 = work.tile([128, 512], BF16, tag="y16")
            nc.vector.tensor_mul(
                y16[:].rearrange("p (t j) -> p t j", j=16),
                e16[:].rearrange("p (t j) -> p t j", j=16),
                wsp[:].unsqueeze(1).broadcast_to([128, 32, 16]),
            )
            # reduce over j (innermost, contiguous); bf16 out feeds the matmul
            with nc.allow_low_precision("sum of 16 bf16 terms; fp32 internal"):
                nc.vector.reduce_sum(
                    sh[:],
                    y16[:].rearrange("p (t j) -> p t j", j=16),
                    axis=mybir.AxisListType.X,
                )
            # cross-partition combine + duplicate on PE
            pt = dup_combine(sh[:], "wfull", 32)
            wsp_update(pt)

            # capture (only steps where a cutoff can occur); pt has full tt
            if s >= smin:
                last_scan_inst = nc.vector.scalar_tensor_tensor(
                    cap_end[:],
                    pt[0:64, END_TAG : END_TAG + 1],
                    hit[:, s : s + 1],
                    cap_end[:],
                    op0=mybir.AluOpType.mult,
                    op1=mybir.AluOpType.add,
                )
            # renorm (logs deferred): every RENORM steps, not at the end
            if (s % RENORM) == 0 and s <= S - 2:
                mx = work.tile([128, 1], F32, tag="mx")
                nc.vector.reduce_max(mx[:], pt[:], axis=mybir.AxisListType.X)
                nc.vector.tensor_copy(mxbuf[:, nren : nren + 1], mx[0:64, :])
                rcp = work.tile([128, 1], F32, tag="rcp")
                nc.vector.reciprocal(rcp[:], mx[:])
                nc.vector.tensor_scalar_mul(wsp[:], wsp[:], rcp[:])
                nren += 1
        assert nren == NR, (nren, NR)

        # ---- tg energy: one fused pass over gathered blocks ----
        tgacc = state.tile([128, 1], F32)
        tgtmp = gathp.tile([128, out_blocks * 64], F32)
        tg_stt = nc.vector.scalar_tensor_tensor(
            tgtmp[:],
            gath[:],
            1.0,
            oh[:],
            op0=mybir.AluOpType.mult,
            op1=mybir.AluOpType.mult,
            accum_out=tgacc[:],
        )
        # The tg pass depends on all 13 dma_gathers (~130us of GpSimd work).
        # Without an explicit ordering dep the scheduler parks it at the HEAD
        # of the Vector queue and the whole scan waits for the gathers
        # (measured: first scan mul at t=138us). Pin it after the scan.
        add_dep_helper(
            tg_stt.ins,
            last_scan_inst.ins,
            sync=False,
            reason="tg pass after scan: avoid head-of-line gather stall",
        )
        # tg halves combined with an f32 dup matmul (precision)
        ptg = psum.tile([128, 1], F32, tag="tg")
        dupf = state.tile([128, 128], F32)
        nc.vector.tensor_copy(dupf[:], dup[:])
        nc.tensor.matmul(ptg[:], dupf[:], tgacc[:], start=True, stop=True)

        # ---- deferred logs + loss assembly ----
        # cap_C[b] = sum_q hcum[b,q] * ln(mxbuf[b,q])
        lnmx = state.tile([64, NR], F32)
        nc.scalar.activation(lnmx[:], mxbuf[:], mybir.ActivationFunctionType.Ln)
        capCtmp = state.tile([64, NR], F32)
        cap_C = state.tile([64, 1], F32)
        nc.vector.scalar_tensor_tensor(
            capCtmp[:],
            lnmx[:],
            1.0,
            hcum[:],
            op0=mybir.AluOpType.mult,
            op1=mybir.AluOpType.mult,
            accum_out=cap_C[:],
        )
        lw = state.tile([64, 1], F32)
        nc.scalar.activation(lw[:], cap_end[:], mybir.ActivationFunctionType.Ln)
        res = state.tile([64, 1], F32)
        nc.vector.tensor_add(res[:], cap_C[:], lw[:])
        nc.vector.tensor_sub(res[:], res[:], ptg[0:64, :])
        nc.sync.dma_start(out_d[:], res[:])

    nc.compile()
    return nc


def host_prep(targets_a: np.ndarray, mask: np.ndarray, S=S):
    """Per-annotator index tensors (pure index arithmetic on targets/mask)."""
    chunks, idx_cols, out_blocks = _plan(S)
    NR = _n_renorms(S)

    tgt = targets_a.astype(np.int64)  # (S, B)
    maskf = mask.astype(np.float32)  # (S, B)
    lens = mask.astype(np.int64).sum(axis=0)  # (B,)
    assert lens.min() >= S // 2, "kernel assumes valid-prefix lens >= S//2"

    hit = np.zeros((64, S), dtype=np.float32)
    hcum = np.zeros((64, NR), dtype=np.float32)
    for b in range(B):
        sb = int(lens[b]) - 1
        hit[b, sb] = 1.0
        win = (sb - 1) // RENORM
        hcum[b, : min(win, NR)] = 1.0

    gidx = np.zeros((128, idx_cols), dtype=np.int16)
    oh = np.zeros((128, out_blocks * 64), dtype=np.float32)
    ohv = oh.reshape(128, out_blocks, 64)
    for (s0, ns, icol, ocol) in chunks:
        ni = ns * B
        i = np.arange(ni)
        sl, bb = np.divmod(i, B)
        rel = (sl * B + bb) * (T * T) + tgt[s0 + sl, bb]
        blk, e = np.divmod(rel, 64)
        gidx[i % 16, icol + i // 16] = blk.astype(np.int16)
        ohv[i % 128, ocol + i // 128, e] = maskf[s0 + sl, bb]
    for g in range(1, 8):
        gidx[16 * g : 16 * (g + 1)] = gidx[:16]

    p = np.arange(128)
    m = np.arange(128)
    dup = ((p[:, None] % 64) == (m[None, :] % 64)).astype(ml_dtypes.bfloat16)

    return dict(gidx=gidx, oh=oh, hit=hit, hcum=hcum, dup=dup)


_NC_CACHE = {}

# Set by test harness to capture profiles; harmless defaults for grading.
TRACE = False
TRACE_DIR = None
LAST_RESULTS = None


def _get_nc(S=S):
    if S not in _NC_CACHE:
        _NC_CACHE[S] = build_nc(S)
    return _NC_CACHE[S]


def kernel(scores, targets, mask, a_mask):
    scores = np.asarray(scores)
    targets = np.asarray(targets)
    mask_np = np.asarray(mask).astype(bool)
    a_mask_np = np.asarray(a_mask).astype(bool)

    nc = _get_nc(scores.shape[1])

    in_maps = []
    for a in range(A):
        prep = host_prep(targets[a], mask_np, S=scores.shape[1])
        m = dict(sc=np.ascontiguousarray(scores[a]), **prep)
        in_maps.append(m)

    if TRACE:
        # Profiling-only: the image's antenv package (imported at boot from a
        # read-only path) lacks axon_hooks; point it at our shim so
        # bass_utils' trace path can find the NTFF hook.
        import antenv

        shim = "/opt/trn_rl_repo/antenv"
        if shim not in list(antenv.__path__):
            antenv.__path__.append(shim)

    global LAST_RESULTS
    res = run_bass_kernel_spmd(
        nc, in_maps, core_ids=list(range(A)), trace=TRACE, tmpdir=TRACE_DIR
    )
    LAST_RESULTS = res
    losses = np.stack([r["losses"][:, 0] for r in res.results])  # (A, B)
    loss = np.where(a_mask_np, losses, 0.0).sum(dtype=np.float32) / np.float32(B)
    return np.float32(loss)


# revision 21
# speedup vs baseline: 1.1945x; 1.1473x over previous
"""CRF loss (multi-annotator) Trainium2 kernel.

Problem (hardcoded): scores (8,200,64,32,32) f32, targets (8,200,64) int,
mask (200,64) bool, a_mask (8,64) bool -> scalar f32 loss.

Sharding: one annotator per NeuronCore (8 cores). Each core computes
losses[b] = log_Z[b] - tg_energy[b] for its annotator; host applies a_mask
and sums / B.

Device algorithm per core:
  - scores stream in s-blocks; SBUF tile rows p = h*64+b (h = tf half),
    free = (j=tf%16, tt) contiguous 2KB rows; exp on ACT reads the (tt,j)
    transposed view and writes (tt,j)-ordered bf16.
  - no-log forward scan, state wsp[h*64+b, j] = exp(p_scan[b, h*16+j] - C[b])
    in bf16. Per step:
      y = exp(scores) * wsp (one TT, wsp broadcast over tt)
      sh = reduce_j y (bf16 tree adds)
      pt(PSUM) = dup128.T @ sh  (PE: adds the two h-halves AND duplicates the
                                 full tt vector into both halves - the only
                                 cross-partition op)
      wsp halves <- pt column slices (one copy on DVE, one on ScalarE)
    Renorm every RENORM steps: m = rowmax(pt) (valid in all rows), wsp *= 1/m,
    m stored to mxbuf; ALL logs deferred to one end-of-kernel pass
    (C prefix sums recovered with a host-built window mask).
  - mask via capture-at-cutoff: hit[s,b] = (s == len_b-1); for s >= S//2-1:
    cap_end += hit * pt[0:64, END_TAG].
  - tg_energy: dma_gather of the 256B block holding each target score,
    then one fused (gather * onehot_mask) row-sum pass; halves combined by
    the same PE duplicate trick.
"""

import os
import sys

import numpy as np

if os.path.isdir("/opt/trn_rl_repo"):
    sys.path.insert(0, "/opt/trn_rl_repo")

import ml_dtypes  # noqa: E402

import concourse.bass as bass  # noqa: E402
import concourse.tile as tile  # noqa: E402
from concourse import bacc, mybir  # noqa: E402
from concourse.tile_rust import add_dep_helper  # noqa: E402
from concourse.bass_utils import run_bass_kernel_spmd  # noqa: E402

F32 = mybir.dt.float32
BF16 = mybir.dt.bfloat16
I16 = mybir.dt.int16

A, S, B, T = 8, 200, 64, 32
START_TAG, END_TAG = 30, 31
SBLK = 8      # steps per streamed DMA block
GBLK = 16     # steps per dma_gather chunk
RENORM = 8    # renorm period (steps)


def _plan(S):
    """Gather chunk plan: list of (s0, nsteps, idx_col0, out_col0)."""
    chunks = []
    s0 = 0
    idx_col = 0
    out_col = 0
    while s0 < S:
        ns = min(GBLK, S - s0)
        ni = ns * B
        assert ni % 128 == 0
        chunks.append((s0, ns, idx_col, out_col))
        idx_col += ni // 16
        out_col += ni // 128
        s0 += ns
    return chunks, idx_col, out_col


def _n_renorms(S):
    # renorms at s = RENORM, 2*RENORM, ... <= S-2
    return max(0, (S - 2) // RENORM)


def build_nc(S=S):
    """Build the Bass/Tile program (same for all cores)."""
    from contextlib import ExitStack

    chunks, idx_cols, out_blocks = _plan(S)
    NR = _n_renorms(S)
    smin = S // 2 - 1  # earliest possible hit step (lens >= S//2)

    nc = bacc.Bacc("TRN2", target_bir_lowering=False, debug=False, num_devices=8)

    sc_d = nc.dram_tensor("sc", [S, B, T, T], F32, kind="ExternalInput").ap()
    gidx_d = nc.dram_tensor("gidx", [128, idx_cols], I16, kind="ExternalInput").ap()
    oh_d = nc.dram_tensor("oh", [128, out_blocks * 64], F32, kind="ExternalInput").ap()
    hit_d = nc.dram_tensor("hit", [64, S], F32, kind="ExternalInput").ap()
    hcum_d = nc.dram_tensor("hcum", [64, NR], F32, kind="ExternalInput").ap()
    dup_d = nc.dram_tensor("dup", [128, 128], BF16, kind="ExternalInput").ap()
    dupf_d = nc.dram_tensor("dupf", [128, 128], F32, kind="ExternalInput").ap()
    out_d = nc.dram_tensor("losses", [64, 1], F32, kind="ExternalOutput").ap()

    with tile.TileContext(nc) as tc, ExitStack() as ctx:
        state = ctx.enter_context(tc.tile_pool(name="state", bufs=1))
        blocks = ctx.enter_context(tc.tile_pool(name="blocks", bufs=3))
        work = ctx.enter_context(tc.tile_pool(name="work", bufs=4))
        gathp = ctx.enter_context(tc.tile_pool(name="gath", bufs=1))
        psum = ctx.enter_context(tc.tile_pool(name="psum", bufs=2, space="PSUM"))

        # ---- persistent state ----
        wsp = state.tile([128, 16], BF16)        # exp(p - C), compact halves
        cap_end = state.tile([64, 1], F32)
        mxbuf = state.tile([64, NR], F32)        # renorm maxes (logs deferred)
        hit = state.tile([64, S], F32)
        hcum = state.tile([64, NR], F32)
        dup = state.tile([128, 128], BF16)
        dupf = state.tile([128, 128], F32)
        gath = gathp.tile([128, out_blocks * 64], F32)
        oh = gathp.tile([128, out_blocks * 64], F32)
        gidx = gathp.tile([128, idx_cols], I16)

        nc.sync.dma_start(dup[:], dup_d[:])
        nc.sync.dma_start(hit[:], hit_d[:])
        nc.sync.dma_start(hcum[:], hcum_d[:])
        nc.sync.dma_start(gidx[:], gidx_d[:])

        nc.vector.memset(cap_end[:], 0.0)

        # ---- gather chunks (overlap with scan) ----
        for (s0, ns, icol, ocol) in chunks:
            ni = ns * B
            src = sc_d[s0 : s0 + ns].rearrange("s b f t -> (s b f t)")
            src_blk = src.rearrange("(n e) -> n e", e=64)
            nc.gpsimd.dma_gather(
                gath[:, ocol * 64 : (ocol + ni // 128) * 64].rearrange(
                    "p (c e) -> p c e", e=64
                ),
                src_blk,
                gidx[:, icol : icol + ni // 16],
                num_idxs=ni,
                num_idxs_reg=ni,
                elem_size=64,
            )

        # ---- streamed score blocks ----
        def load_block(bi):
            s0 = bi * SBLK
            ns = min(SBLK, S - s0)
            blk = blocks.tile([128, SBLK * 512], F32, tag="blk")
            for h in range(2):
                src = (
                    sc_d[s0 : s0 + ns, :, h * 16 : (h + 1) * 16, :]
                    .transpose([1, 0, 2, 3])
                    .rearrange("b s j t -> b s (j t)")
                )
                dst = blk[h * 64 : (h + 1) * 64, 0 : ns * 512].rearrange(
                    "p (s f) -> p s f", s=ns
                )
                nc.sync.dma_start(dst, src)
            return blk

        def dup_combine(sh_ap, tag, n):
            """PSUM <- dup.T @ sh : out[p] = sh[p%64] + sh[64 + p%64]."""
            pt = psum.tile([128, n], F32, tag=tag)
            nc.tensor.matmul(pt[:], dup[:], sh_ap, start=True, stop=True)
            return pt

        def wsp_update(pt):
            # compact halves: wsp[(h,b), j] = pt[(h,b), h*16+j]
            nc.vector.tensor_copy(wsp[0:64, :], pt[0:64, 0:16])
            nc.scalar.copy(wsp[64:128, :], pt[64:128, 16:32])

        blk = load_block(0)

        # ---- init from step 0: p0 = scores[0, :, START_TAG, :] ----
        # START_TAG=30 -> h=1, j=14: rows 64:128, cols 14*32..15*32
        sh = state.tile([128, 32], BF16)  # per-step half sums
        nc.vector.memset(sh[0:64, :], 0.0)
        nc.scalar.activation(
            sh[64:128, :], blk[64:128, 448:480], mybir.ActivationFunctionType.Exp
        )
        pt = dup_combine(sh[:], "wfull", 32)
        wsp_update(pt)

        # ---- main scan ----
        nren = 0
        for s in range(1, S):
            bi, sl = divmod(s, SBLK)
            if sl == 0:
                blk = load_block(bi)
            R = blk[:, sl * 512 : (sl + 1) * 512]
            # exp: read (j,t) storage via (t,j) view; write (t,j)-ordered bf16
            e16 = work.tile([128, 512], BF16, tag="e16")
            nc.scalar.activation(
                e16[:].rearrange("p (t j) -> p t j", j=16),
                R.rearrange("p (j t) -> p t j", j=16),
                mybir.ActivationFunctionType.Exp,
            )
            # y = e * w : wsp row slice broadcast over tt (single 128-row op)
            y16 = work.tile([128, 512], BF16, tag="y16")
            nc.vector.tensor_mul(
                y16[:].rearrange("p (t j) -> p t j", j=16),
                e16[:].rearrange("p (t j) -> p t j", j=16),
                wsp[:].unsqueeze(1).broadcast_to([128, 32, 16]),
            )
            # reduce over j (innermost, contiguous); bf16 out feeds the matmul
            with nc.allow_low_precision("sum of 16 bf16 terms; fp32 internal"):
                nc.vector.reduce_sum(
                    sh[:],
                    y16[:].rearrange("p (t j) -> p t j", j=16),
                    axis=mybir.AxisListType.X,
                )
            # cross-partition combine + duplicate on PE
            pt = dup_combine(sh[:], "wfull", 32)
            wsp_update(pt)

            # capture (only steps where a cutoff can occur); pt has full tt
            if s >= smin:
                last_scan_inst = nc.vector.scalar_tensor_tensor(
                    cap_end[:],
                    pt[0:64, END_TAG : END_TAG + 1],
                    hit[:, s : s + 1],
                    cap_end[:],
                    op0=mybir.AluOpType.mult,
                    op1=mybir.AluOpType.add,
                )
            # renorm (logs deferred): every RENORM steps, not at the end
            if (s % RENORM) == 0 and s <= S - 2:
                mx = work.tile([128, 1], F32, tag="mx")
                nc.vector.reduce_max(mx[:], pt[:], axis=mybir.AxisListType.X)
                nc.vector.tensor_copy(mxbuf[:, nren : nren + 1], mx[0:64, :])
                rcp = work.tile([128, 1], F32, tag="rcp")
                nc.vector.reciprocal(rcp[:], mx[:])
                nc.vector.tensor_scalar_mul(wsp[:], wsp[:], rcp[:])
                nren += 1
        assert nren == NR, (nren, NR)

        # ---- tg energy: one fused pass over gathered blocks ----
        nc.sync.dma_start(oh[:], oh_d[:])
        nc.sync.dma_start(dupf[:], dupf_d[:])
        tgacc = state.tile([128, 1], F32)
        tgtmp = gathp.tile([128, out_blocks * 64], F32)
        tg_stt = nc.vector.scalar_tensor_tensor(
            tgtmp[:],
            gath[:],
            1.0,
            oh[:],
            op0=mybir.AluOpType.mult,
            op1=mybir.AluOpType.mult,
            accum_out=tgacc[:],
        )
        # The tg pass depends on all 13 dma_gathers (~130us of GpSimd work).
        # Without an explicit ordering dep the scheduler parks it at the HEAD
        # of the Vector queue and the whole scan waits for the gathers
        # (measured: first scan mul at t=138us). Pin it after the scan.
        add_dep_helper(
            tg_stt.ins,
            last_scan_inst.ins,
            sync=False,
            reason="tg pass after scan: avoid head-of-line gather stall",
        )
        # tg halves combined with an f32 dup matmul (precision)
        ptg = psum.tile([128, 1], F32, tag="tg")
        nc.tensor.matmul(ptg[:], dupf[:], tgacc[:], start=True, stop=True)

        # ---- deferred logs + loss assembly ----
        # cap_C[b] = sum_q hcum[b,q] * ln(mxbuf[b,q])
        lnmx = state.tile([64, NR], F32)
        nc.scalar.activation(lnmx[:], mxbuf[:], mybir.ActivationFunctionType.Ln)
        capCtmp = state.tile([64, NR], F32)
        cap_C = state.tile([64, 1], F32)
        nc.vector.scalar_tensor_tensor(
            capCtmp[:],
            lnmx[:],
            1.0,
            hcum[:],
            op0=mybir.AluOpType.mult,
            op1=mybir.AluOpType.mult,
            accum_out=cap_C[:],
        )
        lw = state.tile([64, 1], F32)
        nc.scalar.activation(lw[:], cap_end[:], mybir.ActivationFunctionType.Ln)
        res = state.tile([64, 1], F32)
        nc.vector.tensor_add(res[:], cap_C[:], lw[:])
        nc.vector.tensor_sub(res[:], res[:], ptg[0:64, :])
        nc.sync.dma_start(out_d[:], res[:])

    nc.compile()
    return nc


def host_prep(targets_a: np.ndarray, mask: np.ndarray, S=S):
    """Per-annotator index tensors (pure index arithmetic on targets/mask)."""
    chunks, idx_cols, out_blocks = _plan(S)
    NR = _n_renorms(S)

    tgt = targets_a.astype(np.int64)  # (S, B)
    maskf = mask.astype(np.float32)  # (S, B)
    lens = mask.astype(np.int64).sum(axis=0)  # (B,)
    assert lens.min() >= S // 2, "kernel assumes valid-prefix lens >= S//2"

    hit = np.zeros((64, S), dtype=np.float32)
    hcum = np.zeros((64, NR), dtype=np.float32)
    for b in range(B):
        sb = int(lens[b]) - 1
        hit[b, sb] = 1.0
        win = (sb - 1) // RENORM
        hcum[b, : min(win, NR)] = 1.0

    gidx = np.zeros((128, idx_cols), dtype=np.int16)
    oh = np.zeros((128, out_blocks * 64), dtype=np.float32)
    ohv = oh.reshape(128, out_blocks, 64)
    for (s0, ns, icol, ocol) in chunks:
        ni = ns * B
        i = np.arange(ni)
        sl, bb = np.divmod(i, B)
        rel = (sl * B + bb) * (T * T) + tgt[s0 + sl, bb]
        blk, e = np.divmod(rel, 64)
        gidx[i % 16, icol + i // 16] = blk.astype(np.int16)
        ohv[i % 128, ocol + i // 128, e] = maskf[s0 + sl, bb]
    for g in range(1, 8):
        gidx[16 * g : 16 * (g + 1)] = gidx[:16]

    p = np.arange(128)
    m = np.arange(128)
    dupb = (p[:, None] % 64) == (m[None, :] % 64)
    dup = dupb.astype(ml_dtypes.bfloat16)
    dupf = dupb.astype(np.float32)

    return dict(gidx=gidx, oh=oh, hit=hit, hcum=hcum, dup=dup, dupf=dupf)


_NC_CACHE = {}

# Set by test harness to capture profiles; harmless defaults for grading.
TRACE = False
TRACE_DIR = None
LAST_RESULTS = None


def _get_nc(S=S):
    if S not in _NC_CACHE:
        _NC_CACHE[S] = build_nc(S)
    return _NC_CACHE[S]


def kernel(scores, targets, mask, a_mask):
    scores = np.asarray(scores)
    targets = np.asarray(targets)
    mask_np = np.asarray(mask).astype(bool)
    a_mask_np = np.asarray(a_mask).astype(bool)

    nc = _get_nc(scores.shape[1])

    in_maps = []
    for a in range(A):
        prep = host_prep(targets[a], mask_np, S=scores.shape[1])
        m = dict(sc=np.ascontiguousarray(scores[a]), **prep)
        in_maps.append(m)

    if TRACE:
        # Profiling-only: the image's antenv package (imported at boot from a
        # read-only path) lacks axon_hooks; point it at our shim so
        # bass_utils' trace path can find the NTFF hook.
        import antenv

        shim = "/opt/trn_rl_repo/antenv"
        if shim not in list(antenv.__path__):
            antenv.__path__.append(shim)

    global LAST_RESULTS
    res = run_bass_kernel_spmd(
        nc, in_maps, core_ids=list(range(A)), trace=TRACE, tmpdir=TRACE_DIR
    )
    LAST_RESULTS = res
    losses = np.stack([r["losses"][:, 0] for r in res.results])  # (A, B)
    loss = np.where(a_mask_np, losses, 0.0).sum(dtype=np.float32) / np.float32(B)
    return np.float32(loss)
